# revision 29
# baseline (speedup 1.0000x reference)
"""KLDivLoss(batchmean) of softmax(f1_rewards/tau) against log(output).

Contract: kernel(output=[1024,4096,1] f32, labels=[1024,4096] i32) -> () f32.

Math (per batch row):
    c_k  = cumsum(labels)            (k = 1..L)
    T    = c_L
    s'_k = c_k / (k + T)             (true s = (2/tau)*s'; the reference's
                                      where() guards collapse: c_k=0 => s=0.
                                      s in [0, ~1.18] -> exp safe without
                                      max-subtraction)
    q    = softmax((2/tau)*s');  Z = sum exp;  log q = s - ln Z
    row  = (2/tau)*sum_k q_k*s'_k - ln Z - sum_k q_k*ln p_k
    loss = sum_rows(row) / B

Distribution: pure data-parallel, 128 batch rows per NeuronCore (= the 128
SBUF partitions), 8 cores. Each core emits one f32 partial (its row-sum);
the host adds the 8 partials and divides by B.

Per-core structure:
  - labels stream on one DMA queue (sync), p on another (tensor) so the
    scan chain starts as early as possible
  - independent local cumsum per chunk; carry offsets come from one tiny
    scan of chunk totals and ride the s-computation's scalar slot for free
  - kT = iota+T (2x tensor_scalar), inv = reciprocal_approx_fast,
    s' = (c_local+off)*inv, e = exp((2/tau)s') with free per-chunk Z
  - row-dots sum q*s' and sum q*ln p: chunks 0..1 via DVE affine_mul_reduce
    ((x*invZ)*e with free row-accumulate) in parallel with chunks 2..3 on
    PE (fp16 diagonal-block matmuls of q = e*invZ against s'/lp windows)
  - final partition sum via a [128,1] ones-matmul on PE
"""

import numpy as np

B, L = 1024, 4096
N_CORES = 8
RPC = B // N_CORES  # rows per core = 128 = SBUF partitions
TAU = 0.85
CH = 1024  # free-dim chunk
NCH = L // CH
MM = 128  # matmul window
WPC = CH // MM
DVE_CHUNKS = (0, 1)  # contraction on DVE (amr); the rest go to PE

_NC_CACHE = {}


def build_nc():
    import concourse.bacc as bacc
    import concourse.mybir as mybir
    import concourse.tile as tile

    f32 = mybir.dt.float32
    f16 = mybir.dt.float16
    i8 = mybir.dt.int8
    Alu = mybir.AluOpType
    Act = mybir.ActivationFunctionType
    Ax = mybir.AxisListType

    nc = bacc.Bacc(
        "TRN2", target_bir_lowering=False, debug=False, num_devices=N_CORES
    )
    labels_d = nc.dram_tensor("labels", [RPC, L], i8, kind="ExternalInput").ap()
    p_d = nc.dram_tensor("p", [RPC, L], f32, kind="ExternalInput").ap()
    out_d = nc.dram_tensor("partial", [1, 1], f32, kind="ExternalOutput").ap()

    pe_chunks = tuple(j for j in range(NCH) if j not in DVE_CHUNKS)
    pe_nwin = len(pe_chunks) * WPC

    with tile.TileContext(nc) as tc:
        with (
            tc.tile_pool(name="persist", bufs=1) as persist,
            tc.tile_pool(name="lab", bufs=3) as lab_pool,
            tc.tile_pool(name="pin", bufs=3) as p_pool,
            tc.tile_pool(name="tmp", bufs=2) as tmp_pool,
            tc.tile_pool(name="small", bufs=1) as small,
            tc.tile_pool(name="psum", bufs=1, space="PSUM") as psum_pool,
        ):
            iota_t = persist.tile([RPC, L], mybir.dt.int32)
            nc.gpsimd.iota(
                iota_t[:], pattern=[[1, L]], base=1, channel_multiplier=0
            )
            ident = persist.tile([MM, MM], f32)
            nc.gpsimd.memset(ident[:], 1.0)
            nc.gpsimd.affine_select(
                ident[:],
                ident[:],
                pattern=[[-1, MM]],
                compare_op=Alu.is_equal,
                fill=0.0,
                base=0,
                channel_multiplier=1,
            )
            ones_col = persist.tile([RPC, 1], f32)
            nc.gpsimd.memset(ones_col[:], 1.0)

            c_full = persist.tile([RPC, L], f32)
            e_full = persist.tile([RPC, L], f16)
            s16 = persist.tile([RPC, L], f16)
            lp16 = persist.tile([RPC, L], f16)
            Zc = small.tile([RPC, NCH], f32)

            # Phase 1: int8 labels stream first (sync queue), independent
            # local cumsum per chunk; then p (scalar queue) with ln -> fp16.
            labs = []
            for j in range(NCH):
                sl = slice(j * CH, (j + 1) * CH)
                lab = lab_pool.tile([RPC, CH], i8, tag="lab")
                nc.sync.dma_start(lab[:], labels_d[:, sl])
                labs.append(lab)
            for j in range(NCH):
                sl = slice(j * CH, (j + 1) * CH)
                nc.vector.tensor_tensor_scan(
                    c_full[:, sl], labs[j][:], labs[j][:], 0.0, Alu.add,
                    Alu.bypass,
                )
            # p DMAs queue on sync BEHIND the labels so labels get full
            # bandwidth first. ln(p) for all but the last chunk; the last Ln
            # is emitted after the kT activations so kT0 isn't stuck behind
            # it on the ACT queue.
            for j in range(NCH):
                sl = slice(j * CH, (j + 1) * CH)
                pt = p_pool.tile([RPC, CH], f32, tag="p")
                nc.sync.dma_start(pt[:], p_d[:, sl])
                if j < NCH - 1:
                    nc.scalar.activation(lp16[:, sl], pt[:], Act.Ln)
                else:
                    last_pt = pt

            # chunk offsets: tiny scan over the strided chunk-total column
            offs = small.tile([RPC, NCH], f32)
            tot_view = c_full[:, CH - 1 :: CH]
            nc.vector.tensor_tensor_scan(
                offs[:], tot_view, tot_view, 0.0, Alu.add, Alu.bypass
            )
            T_ap = offs[:, NCH - 1 : NCH]
            # (tau/2)*T so ACT computes kT' = (tau/2)*(k+T); then
            # inv = 1/kT' = (2/tau)/(k+T) and s16 holds the TRUE s.
            T_scaled = small.tile([RPC, 1], f32)
            nc.vector.tensor_scalar(
                T_scaled[:], T_ap, TAU / 2.0, None, Alu.mult
            )

            # Phase 2: kT = iota + T on ACT (Identity with per-row bias);
            # inv = 1/(k+T); s' = (c_local+off)*inv (fp16);
            # e = exp((2/tau)*s') with per-chunk Z accumulate;
            # d = (2/tau)*s' - ln p (fp16, 2x mode) for the single tail
            # contraction sum q*d = sum q*s - sum q*ln p.
            d16 = persist.tile([RPC, L], f16)
            kTs = []
            for j in range(NCH):
                kT = tmp_pool.tile([RPC, CH], f32, tag="kT")
                sl = slice(j * CH, (j + 1) * CH)
                if j == 0:
                    # DVE so the pipeline isn't gated on the ACT queue
                    nc.vector.tensor_scalar(
                        kT[:], iota_t[:, sl], T_ap, TAU / 2.0,
                        Alu.add, Alu.mult,
                    )
                else:
                    nc.scalar.activation(
                        kT[:], iota_t[:, sl], Act.Identity,
                        bias=T_scaled[:], scale=TAU / 2.0,
                    )
                kTs.append(kT)
            nc.scalar.activation(
                lp16[:, (NCH - 1) * CH :], last_pt[:], Act.Ln
            )
            for j in range(NCH):
                sl = slice(j * CH, (j + 1) * CH)
                inv = tmp_pool.tile([RPC, CH], f32, tag="inv")
                nc.vector.reciprocal_approx_fast(inv[:], kTs[j][:])
                off = 0.0 if j == 0 else offs[:, j - 1 : j]
                nc.vector.scalar_tensor_tensor(
                    s16[:, sl], c_full[:, sl], off, inv[:], Alu.add, Alu.mult
                )
                nc.scalar.activation(
                    e_full[:, sl],
                    s16[:, sl],
                    Act.Exp,
                    accum_out=Zc[:, j : j + 1],
                )
            # d = s - ln p is only needed by the tail contractions; emit the
            # subtractions after the s/exp chain so Z is reached sooner.
            for j in range(NCH):
                sl = slice(j * CH, (j + 1) * CH)
                nc.vector.tensor_sub(d16[:, sl], s16[:, sl], lp16[:, sl])

            Z = small.tile([RPC, 1], f32)
            nc.vector.tensor_reduce(Z[:], Zc[:], Ax.X, Alu.add)
            invZ = small.tile([RPC, 1], f32)
            nc.vector.reciprocal_approx_fast(invZ[:], Z[:])
            lnZ = small.tile([RPC, 1], f32)
            nc.scalar.activation(lnZ[:], Z[:], Act.Ln)

            # Phase 3: PE chunks first (eps feed the matmul stream), then
            # DVE chunks via affine_mul_reduce on d = (2/tau)s' - lp.
            psum_d = psum_pool.tile([MM, MM], f32, tag="pd")
            g = 0
            for j in pe_chunks:
                sl = slice(j * CH, (j + 1) * CH)
                ep = tmp_pool.tile([RPC, CH], f16, tag="ep")
                nc.vector.tensor_scalar(
                    ep[:], e_full[:, sl], invZ[:], None, Alu.mult
                )
                for w in range(WPC):
                    wsl = slice(j * CH + w * MM, j * CH + (w + 1) * MM)
                    nc.tensor.matmul(
                        psum_d[:], ep[:, w * MM : (w + 1) * MM], d16[:, wsl],
                        start=(g == 0), stop=(g == pe_nwin - 1),
                    )
                    g += 1

            ABc = small.tile([RPC, len(DVE_CHUNKS)], f32)
            for idx, j in enumerate(DVE_CHUNKS):
                sl = slice(j * CH, (j + 1) * CH)
                scr_d = tmp_pool.tile([RPC, CH], f32, tag="scrd")
                nc.vector.affine_mul_reduce(
                    scr_d[:], ABc[:, idx : idx + 1], d16[:, sl], e_full[:, sl],
                    invZ[:], 0.0,
                )

            scr_dd = small.tile([MM, MM], f32)
            diag_d = small.tile([MM, 1], f32)
            nc.vector.scalar_tensor_tensor(
                scr_dd[:], psum_d[:], 1.0, ident[:], Alu.mult, Alu.mult,
                accum_out=diag_d[:],
            )

            # u = (diag_d + sum(ABc)) - lnZ; DMA per-row u, host sums
            ABdve = small.tile([RPC, 1], f32)
            nc.vector.tensor_reduce(ABdve[:], ABc[:], Ax.X, Alu.add)
            u0 = small.tile([RPC, 1], f32)
            nc.vector.tensor_add(u0[:], diag_d[:], ABdve[:])
            u = small.tile([RPC, 1], f32)
            nc.vector.tensor_sub(u[:], u0[:], lnZ[:])
            psum_u = psum_pool.tile([1, 1], f32, tag="pu")
            nc.tensor.matmul(
                psum_u[:], u[:], ones_col[:], start=True, stop=True
            )
            res = small.tile([1, 1], f32)
            nc.vector.tensor_copy(res[:], psum_u[:])
            nc.sync.dma_start(out_d[:, :], res[:])

    # Steer the ACT-table chooser to the one set containing BOTH exp and
    # ln so the kernel pays a single ACT_TABLE_LOAD instead of four.
    orig_tables = bacc.get_activation_tables
    combined = "natural_log_exp_and_others"

    def _patched_tables(arch):
        t = orig_tables(arch)
        if combined in t:
            for name, funcs in t.items():
                if name != combined:
                    funcs.discard(Act.Exp)
                    funcs.discard(Act.Ln)
        return t

    bacc.get_activation_tables = _patched_tables
    try:
        nc.compile()
    finally:
        bacc.get_activation_tables = orig_tables
    return nc


def get_nc():
    nc = _NC_CACHE.get("nc")
    if nc is None:
        nc = build_nc()
        _NC_CACHE["nc"] = nc
    return nc


def shard_inputs(output, labels):
    p = np.ascontiguousarray(
        np.asarray(output, dtype=np.float32).reshape(B, L)
    )
    # labels are 0/1 -> int8 is a lossless re-encoding, 4x less HBM traffic
    lab = np.ascontiguousarray(np.asarray(labels).astype(np.int8))
    return [
        {
            "labels": lab[i * RPC : (i + 1) * RPC],
            "p": p[i * RPC : (i + 1) * RPC],
        }
        for i in range(N_CORES)
    ]


def gather(results):
    total = np.float64(0.0)
    for r in results:
        total += np.float64(r["partial"].reshape(-1)[0])
    return np.array(total / B, dtype=np.float32)


def kernel(output, labels):
    from concourse.bass_utils import run_bass_kernel_spmd

    nc = get_nc()
    in_maps = shard_inputs(output, labels)
    res = run_bass_kernel_spmd(nc, in_maps, list(range(N_CORES)))
    return gather(res.results)


# revision 31
# speedup vs baseline: 1.0042x; 1.0042x over previous
"""KLDivLoss(batchmean) of softmax(f1_rewards/tau) against log(output).

Contract: kernel(output=[1024,4096,1] f32, labels=[1024,4096] i32) -> () f32.

Math (per batch row):
    c_k  = cumsum(labels)            (k = 1..L)
    T    = c_L
    s'_k = c_k / (k + T)             (true s = (2/tau)*s'; the reference's
                                      where() guards collapse: c_k=0 => s=0.
                                      s in [0, ~1.18] -> exp safe without
                                      max-subtraction)
    q    = softmax((2/tau)*s');  Z = sum exp;  log q = s - ln Z
    row  = (2/tau)*sum_k q_k*s'_k - ln Z - sum_k q_k*ln p_k
    loss = sum_rows(row) / B

Distribution: pure data-parallel, 128 batch rows per NeuronCore (= the 128
SBUF partitions), 8 cores. Each core emits one f32 partial (its row-sum);
the host adds the 8 partials and divides by B.

Per-core structure:
  - labels stream on one DMA queue (sync), p on another (tensor) so the
    scan chain starts as early as possible
  - independent local cumsum per chunk; carry offsets come from one tiny
    scan of chunk totals and ride the s-computation's scalar slot for free
  - kT = iota+T (2x tensor_scalar), inv = reciprocal_approx_fast,
    s' = (c_local+off)*inv, e = exp((2/tau)s') with free per-chunk Z
  - row-dots sum q*s' and sum q*ln p: chunks 0..1 via DVE affine_mul_reduce
    ((x*invZ)*e with free row-accumulate) in parallel with chunks 2..3 on
    PE (fp16 diagonal-block matmuls of q = e*invZ against s'/lp windows)
  - final partition sum via a [128,1] ones-matmul on PE
"""

import numpy as np

B, L = 1024, 4096
N_CORES = 8
RPC = B // N_CORES  # rows per core = 128 = SBUF partitions
TAU = 0.85
CH = 1024  # free-dim chunk
NCH = L // CH
MM = 128  # matmul window
WPC = CH // MM
DVE_CHUNKS = (0, 1)  # contraction on DVE (amr); the rest go to PE

_NC_CACHE = {}


def build_nc():
    import concourse.bacc as bacc
    import concourse.mybir as mybir
    import concourse.tile as tile

    f32 = mybir.dt.float32
    f16 = mybir.dt.float16
    i8 = mybir.dt.int8
    Alu = mybir.AluOpType
    Act = mybir.ActivationFunctionType
    Ax = mybir.AxisListType

    nc = bacc.Bacc(
        "TRN2", target_bir_lowering=False, debug=False, num_devices=N_CORES
    )
    labels_d = nc.dram_tensor("labels", [RPC, L], i8, kind="ExternalInput").ap()
    p_d = nc.dram_tensor("p", [RPC, L], f32, kind="ExternalInput").ap()
    out_d = nc.dram_tensor("partial", [1, 1], f32, kind="ExternalOutput").ap()

    pe_chunks = tuple(j for j in range(NCH) if j not in DVE_CHUNKS)
    pe_nwin = len(pe_chunks) * WPC

    with tile.TileContext(nc) as tc:
        with (
            tc.tile_pool(name="persist", bufs=1) as persist,
            tc.tile_pool(name="lab", bufs=3) as lab_pool,
            tc.tile_pool(name="pin", bufs=3) as p_pool,
            tc.tile_pool(name="tmp", bufs=2) as tmp_pool,
            tc.tile_pool(name="small", bufs=1) as small,
            tc.tile_pool(name="psum", bufs=1, space="PSUM") as psum_pool,
        ):
            iota_t = persist.tile([RPC, L], mybir.dt.int32)
            nc.gpsimd.iota(
                iota_t[:], pattern=[[1, L]], base=1, channel_multiplier=0
            )
            ident = persist.tile([MM, MM], f32)
            nc.gpsimd.memset(ident[:], 1.0)
            nc.gpsimd.affine_select(
                ident[:],
                ident[:],
                pattern=[[-1, MM]],
                compare_op=Alu.is_equal,
                fill=0.0,
                base=0,
                channel_multiplier=1,
            )
            ones_col = persist.tile([RPC, 1], f32)
            nc.gpsimd.memset(ones_col[:], 1.0)

            c_full = persist.tile([RPC, L], f32)
            e_full = persist.tile([RPC, L], f32)
            s16 = persist.tile([RPC, L], f16)
            lp16 = persist.tile([RPC, L], f16)
            Zc = small.tile([RPC, NCH], f32)

            # Phase 1: int8 labels stream first (sync queue), independent
            # local cumsum per chunk; then p (scalar queue) with ln -> fp16.
            labs = []
            for j in range(NCH):
                sl = slice(j * CH, (j + 1) * CH)
                lab = lab_pool.tile([RPC, CH], i8, tag="lab")
                # alternate queues so label chunks transfer in parallel
                eng = nc.sync if j % 2 == 0 else nc.scalar
                eng.dma_start(lab[:], labels_d[:, sl])
                labs.append(lab)
            for j in range(NCH):
                sl = slice(j * CH, (j + 1) * CH)
                nc.vector.tensor_tensor_scan(
                    c_full[:, sl], labs[j][:], labs[j][:], 0.0, Alu.add,
                    Alu.bypass,
                )
            # p DMAs queue on sync BEHIND the labels so labels get full
            # bandwidth first. ln(p) for all but the last chunk; the last Ln
            # is emitted after the kT activations so kT0 isn't stuck behind
            # it on the ACT queue.
            for j in range(NCH):
                sl = slice(j * CH, (j + 1) * CH)
                pt = p_pool.tile([RPC, CH], f32, tag="p")
                nc.sync.dma_start(pt[:], p_d[:, sl])
                if j < NCH - 1:
                    nc.scalar.activation(lp16[:, sl], pt[:], Act.Ln)
                else:
                    last_pt = pt

            # chunk offsets: tiny scan over the strided chunk-total column
            offs = small.tile([RPC, NCH], f32)
            tot_view = c_full[:, CH - 1 :: CH]
            nc.vector.tensor_tensor_scan(
                offs[:], tot_view, tot_view, 0.0, Alu.add, Alu.bypass
            )
            T_ap = offs[:, NCH - 1 : NCH]
            # (tau/2)*T so ACT computes kT' = (tau/2)*(k+T); then
            # inv = 1/kT' = (2/tau)/(k+T) and s16 holds the TRUE s.
            T_scaled = small.tile([RPC, 1], f32)
            nc.vector.tensor_scalar(
                T_scaled[:], T_ap, TAU / 2.0, None, Alu.mult
            )

            # Phase 2: kT = iota + T on ACT (Identity with per-row bias);
            # inv = 1/(k+T); s' = (c_local+off)*inv (fp16);
            # e = exp((2/tau)*s') with per-chunk Z accumulate;
            # d = (2/tau)*s' - ln p (fp16, 2x mode) for the single tail
            # contraction sum q*d = sum q*s - sum q*ln p.
            d16 = persist.tile([RPC, L], f16)
            kTs = []
            for j in range(NCH):
                kT = tmp_pool.tile([RPC, CH], f32, tag="kT")
                sl = slice(j * CH, (j + 1) * CH)
                if j == 0:
                    # DVE so the pipeline isn't gated on the ACT queue
                    nc.vector.tensor_scalar(
                        kT[:], iota_t[:, sl], T_ap, TAU / 2.0,
                        Alu.add, Alu.mult,
                    )
                else:
                    nc.scalar.activation(
                        kT[:], iota_t[:, sl], Act.Identity,
                        bias=T_scaled[:], scale=TAU / 2.0,
                    )
                kTs.append(kT)
            nc.scalar.activation(
                lp16[:, (NCH - 1) * CH :], last_pt[:], Act.Ln
            )
            for j in range(NCH):
                sl = slice(j * CH, (j + 1) * CH)
                inv = tmp_pool.tile([RPC, CH], f32, tag="inv")
                nc.vector.reciprocal_approx_fast(inv[:], kTs[j][:])
                off = 0.0 if j == 0 else offs[:, j - 1 : j]
                nc.vector.scalar_tensor_tensor(
                    s16[:, sl], c_full[:, sl], off, inv[:], Alu.add, Alu.mult
                )
                nc.scalar.activation(
                    e_full[:, sl],
                    s16[:, sl],
                    Act.Exp,
                    accum_out=Zc[:, j : j + 1],
                )
            # d = s - ln p is only needed by the tail contractions; emit the
            # subtractions after the s/exp chain so Z is reached sooner.
            for j in range(NCH):
                sl = slice(j * CH, (j + 1) * CH)
                nc.vector.tensor_sub(d16[:, sl], s16[:, sl], lp16[:, sl])

            Z = small.tile([RPC, 1], f32)
            nc.vector.tensor_reduce(Z[:], Zc[:], Ax.X, Alu.add)
            invZ = small.tile([RPC, 1], f32)
            nc.vector.reciprocal_approx_fast(invZ[:], Z[:])
            lnZ = small.tile([RPC, 1], f32)
            nc.scalar.activation(lnZ[:], Z[:], Act.Ln)

            # Phase 3: PE chunks first (eps feed the matmul stream), then
            # DVE chunks via affine_mul_reduce on d = (2/tau)s' - lp.
            psum_d = psum_pool.tile([MM, MM], f32, tag="pd")
            g = 0
            for j in pe_chunks:
                sl = slice(j * CH, (j + 1) * CH)
                ep = tmp_pool.tile([RPC, CH], f16, tag="ep")
                nc.vector.tensor_scalar(
                    ep[:], e_full[:, sl], invZ[:], None, Alu.mult
                )
                for w in range(WPC):
                    wsl = slice(j * CH + w * MM, j * CH + (w + 1) * MM)
                    nc.tensor.matmul(
                        psum_d[:], ep[:, w * MM : (w + 1) * MM], d16[:, wsl],
                        start=(g == 0), stop=(g == pe_nwin - 1),
                    )
                    g += 1

            ABc = small.tile([RPC, len(DVE_CHUNKS)], f32)
            for idx, j in enumerate(DVE_CHUNKS):
                sl = slice(j * CH, (j + 1) * CH)
                scr_d = tmp_pool.tile([RPC, CH], f32, tag="scrd")
                nc.vector.affine_mul_reduce(
                    scr_d[:], ABc[:, idx : idx + 1], d16[:, sl], e_full[:, sl],
                    invZ[:], 0.0,
                )

            scr_dd = small.tile([MM, MM], f32)
            diag_d = small.tile([MM, 1], f32)
            nc.vector.scalar_tensor_tensor(
                scr_dd[:], psum_d[:], 1.0, ident[:], Alu.mult, Alu.mult,
                accum_out=diag_d[:],
            )

            # u = (diag_d + sum(ABc)) - lnZ; DMA per-row u, host sums
            ABdve = small.tile([RPC, 1], f32)
            nc.vector.tensor_reduce(ABdve[:], ABc[:], Ax.X, Alu.add)
            u0 = small.tile([RPC, 1], f32)
            nc.vector.tensor_add(u0[:], diag_d[:], ABdve[:])
            u = small.tile([RPC, 1], f32)
            nc.vector.tensor_sub(u[:], u0[:], lnZ[:])
            psum_u = psum_pool.tile([1, 1], f32, tag="pu")
            nc.tensor.matmul(
                psum_u[:], u[:], ones_col[:], start=True, stop=True
            )
            res = small.tile([1, 1], f32)
            nc.vector.tensor_copy(res[:], psum_u[:])
            nc.sync.dma_start(out_d[:, :], res[:])

    # Steer the ACT-table chooser to the one set containing BOTH exp and
    # ln so the kernel pays a single ACT_TABLE_LOAD instead of four.
    orig_tables = bacc.get_activation_tables
    combined = "natural_log_exp_and_others"

    def _patched_tables(arch):
        t = orig_tables(arch)
        if combined in t:
            for name, funcs in t.items():
                if name != combined:
                    funcs.discard(Act.Exp)
                    funcs.discard(Act.Ln)
        return t

    bacc.get_activation_tables = _patched_tables
    try:
        nc.compile()
    finally:
        bacc.get_activation_tables = orig_tables
    return nc


def get_nc():
    nc = _NC_CACHE.get("nc")
    if nc is None:
        nc = build_nc()
        _NC_CACHE["nc"] = nc
    return nc


def shard_inputs(output, labels):
    p = np.ascontiguousarray(
        np.asarray(output, dtype=np.float32).reshape(B, L)
    )
    # labels are 0/1 -> int8 is a lossless re-encoding, 4x less HBM traffic
    lab = np.ascontiguousarray(np.asarray(labels).astype(np.int8))
    return [
        {
            "labels": lab[i * RPC : (i + 1) * RPC],
            "p": p[i * RPC : (i + 1) * RPC],
        }
        for i in range(N_CORES)
    ]


def gather(results):
    total = np.float64(0.0)
    for r in results:
        total += np.float64(r["partial"].reshape(-1)[0])
    return np.array(total / B, dtype=np.float32)


def kernel(output, labels):
    from concourse.bass_utils import run_bass_kernel_spmd

    nc = get_nc()
    in_maps = shard_inputs(output, labels)
    res = run_bass_kernel_spmd(nc, in_maps, list(range(N_CORES)))
    return gather(res.results)


# revision 32
# speedup vs baseline: 1.0260x; 1.0217x over previous
"""KLDivLoss(batchmean) of softmax(f1_rewards/tau) against log(output).

Contract: kernel(output=[1024,4096,1] f32, labels=[1024,4096] i32) -> () f32.

Math (per batch row):
    c_k  = cumsum(labels)            (k = 1..L)
    T    = c_L
    s'_k = c_k / (k + T)             (true s = (2/tau)*s'; the reference's
                                      where() guards collapse: c_k=0 => s=0.
                                      s in [0, ~1.18] -> exp safe without
                                      max-subtraction)
    q    = softmax((2/tau)*s');  Z = sum exp;  log q = s - ln Z
    row  = (2/tau)*sum_k q_k*s'_k - ln Z - sum_k q_k*ln p_k
    loss = sum_rows(row) / B

Distribution: pure data-parallel, 128 batch rows per NeuronCore (= the 128
SBUF partitions), 8 cores. Each core emits one f32 partial (its row-sum);
the host adds the 8 partials and divides by B.

Per-core structure:
  - labels stream on one DMA queue (sync), p on another (tensor) so the
    scan chain starts as early as possible
  - independent local cumsum per chunk; carry offsets come from one tiny
    scan of chunk totals and ride the s-computation's scalar slot for free
  - kT = iota+T (2x tensor_scalar), inv = reciprocal_approx_fast,
    s' = (c_local+off)*inv, e = exp((2/tau)s') with free per-chunk Z
  - row-dots sum q*s' and sum q*ln p: chunks 0..1 via DVE affine_mul_reduce
    ((x*invZ)*e with free row-accumulate) in parallel with chunks 2..3 on
    PE (fp16 diagonal-block matmuls of q = e*invZ against s'/lp windows)
  - final partition sum via a [128,1] ones-matmul on PE
"""

import numpy as np

B, L = 1024, 4096
N_CORES = 8
RPC = B // N_CORES  # rows per core = 128 = SBUF partitions
TAU = 0.85
CH = 1024  # free-dim chunk
NCH = L // CH
MM = 128  # matmul window
WPC = CH // MM
DVE_CHUNKS = (0, 1)  # contraction on DVE (amr); the rest go to PE

_NC_CACHE = {}


def build_nc():
    import concourse.bacc as bacc
    import concourse.mybir as mybir
    import concourse.tile as tile

    f32 = mybir.dt.float32
    f16 = mybir.dt.float16
    i8 = mybir.dt.int8
    Alu = mybir.AluOpType
    Act = mybir.ActivationFunctionType
    Ax = mybir.AxisListType

    nc = bacc.Bacc(
        "TRN2", target_bir_lowering=False, debug=False, num_devices=N_CORES
    )
    labels_d = nc.dram_tensor("labels", [RPC, L], i8, kind="ExternalInput").ap()
    p_d = nc.dram_tensor("p", [RPC, L], f32, kind="ExternalInput").ap()
    out_d = nc.dram_tensor("partial", [1, 1], f32, kind="ExternalOutput").ap()

    pe_chunks = tuple(j for j in range(NCH) if j not in DVE_CHUNKS)
    pe_nwin = len(pe_chunks) * WPC

    with tile.TileContext(nc) as tc:
        with (
            tc.tile_pool(name="persist", bufs=1) as persist,
            tc.tile_pool(name="lab", bufs=3) as lab_pool,
            tc.tile_pool(name="pin", bufs=3) as p_pool,
            tc.tile_pool(name="tmp", bufs=2) as tmp_pool,
            tc.tile_pool(name="small", bufs=1) as small,
            tc.tile_pool(name="psum", bufs=1, space="PSUM") as psum_pool,
        ):
            iota_t = persist.tile([RPC, L], mybir.dt.int32)
            nc.gpsimd.iota(
                iota_t[:], pattern=[[1, L]], base=1, channel_multiplier=0
            )
            ident = persist.tile([MM, MM], f32)
            nc.gpsimd.memset(ident[:], 1.0)
            nc.gpsimd.affine_select(
                ident[:],
                ident[:],
                pattern=[[-1, MM]],
                compare_op=Alu.is_equal,
                fill=0.0,
                base=0,
                channel_multiplier=1,
            )
            ones_col = persist.tile([RPC, 1], f32)
            nc.gpsimd.memset(ones_col[:], 1.0)

            c_full = persist.tile([RPC, L], f32)
            e_full = persist.tile([RPC, L], f32)
            s16 = persist.tile([RPC, L], f16)
            lp16 = persist.tile([RPC, L], f16)
            Zc = small.tile([RPC, NCH], f32)

            # Phase 1: int8 labels stream first (sync queue), independent
            # local cumsum per chunk; then p (scalar queue) with ln -> fp16.
            labs = []
            for j in range(NCH):
                sl = slice(j * CH, (j + 1) * CH)
                lab = lab_pool.tile([RPC, CH], i8, tag="lab")
                nc.sync.dma_start(lab[:], labels_d[:, sl])
                labs.append(lab)
            for j in range(NCH):
                sl = slice(j * CH, (j + 1) * CH)
                nc.vector.tensor_tensor_scan(
                    c_full[:, sl], labs[j][:], labs[j][:], 0.0, Alu.add,
                    Alu.bypass,
                )
            # p DMAs queue on sync BEHIND the labels so labels get full
            # bandwidth first. ln(p) for all but the last chunk; the last Ln
            # is emitted after the kT activations so kT0 isn't stuck behind
            # it on the ACT queue.
            for j in range(NCH):
                sl = slice(j * CH, (j + 1) * CH)
                pt = p_pool.tile([RPC, CH], f32, tag="p")
                nc.sync.dma_start(pt[:], p_d[:, sl])
                if j < NCH - 1:
                    nc.scalar.activation(lp16[:, sl], pt[:], Act.Ln)
                else:
                    last_pt = pt

            # chunk offsets: tiny scan over the strided chunk-total column
            offs = small.tile([RPC, NCH], f32)
            tot_view = c_full[:, CH - 1 :: CH]
            nc.vector.tensor_tensor_scan(
                offs[:], tot_view, tot_view, 0.0, Alu.add, Alu.bypass
            )
            T_ap = offs[:, NCH - 1 : NCH]
            # (tau/2)*T so ACT computes kT' = (tau/2)*(k+T); then
            # inv = 1/kT' = (2/tau)/(k+T) and s16 holds the TRUE s.
            T_scaled = small.tile([RPC, 1], f32)
            nc.vector.tensor_scalar(
                T_scaled[:], T_ap, TAU / 2.0, None, Alu.mult
            )

            # Phase 2: kT = iota + T on ACT (Identity with per-row bias);
            # inv = 1/(k+T); s' = (c_local+off)*inv (fp16);
            # e = exp((2/tau)*s') with per-chunk Z accumulate;
            # d = (2/tau)*s' - ln p (fp16, 2x mode) for the single tail
            # contraction sum q*d = sum q*s - sum q*ln p.
            d16 = persist.tile([RPC, L], f16)
            kTs = []
            for j in range(NCH):
                kT = tmp_pool.tile([RPC, CH], f32, tag="kT")
                sl = slice(j * CH, (j + 1) * CH)
                if j == 0:
                    # DVE so the pipeline isn't gated on the ACT queue
                    nc.vector.tensor_scalar(
                        kT[:], iota_t[:, sl], T_ap, TAU / 2.0,
                        Alu.add, Alu.mult,
                    )
                else:
                    nc.scalar.activation(
                        kT[:], iota_t[:, sl], Act.Identity,
                        bias=T_scaled[:], scale=TAU / 2.0,
                    )
                kTs.append(kT)
            nc.scalar.activation(
                lp16[:, (NCH - 1) * CH :], last_pt[:], Act.Ln
            )
            for j in range(NCH):
                sl = slice(j * CH, (j + 1) * CH)
                inv = tmp_pool.tile([RPC, CH], f32, tag="inv")
                nc.vector.reciprocal_approx_fast(inv[:], kTs[j][:])
                off = 0.0 if j == 0 else offs[:, j - 1 : j]
                nc.vector.scalar_tensor_tensor(
                    s16[:, sl], c_full[:, sl], off, inv[:], Alu.add, Alu.mult
                )
                nc.scalar.activation(
                    e_full[:, sl],
                    s16[:, sl],
                    Act.Exp,
                    accum_out=Zc[:, j : j + 1],
                )
            # d = s - ln p is only needed by the tail contractions; emit the
            # subtractions after the s/exp chain so Z is reached sooner.
            for j in range(NCH):
                sl = slice(j * CH, (j + 1) * CH)
                nc.vector.tensor_sub(d16[:, sl], s16[:, sl], lp16[:, sl])

            Z = small.tile([RPC, 1], f32)
            nc.vector.tensor_reduce(Z[:], Zc[:], Ax.X, Alu.add)
            invZ = small.tile([RPC, 1], f32)
            nc.vector.reciprocal_approx_fast(invZ[:], Z[:])
            lnZ = small.tile([RPC, 1], f32)
            nc.scalar.activation(lnZ[:], Z[:], Act.Ln)

            # Phase 3: PE chunks first (eps feed the matmul stream), then
            # DVE chunks via affine_mul_reduce on d = (2/tau)s' - lp.
            psum_d = psum_pool.tile([MM, MM], f32, tag="pd")
            g = 0
            for j in pe_chunks:
                sl = slice(j * CH, (j + 1) * CH)
                ep = tmp_pool.tile([RPC, CH], f16, tag="ep")
                nc.vector.tensor_scalar(
                    ep[:], e_full[:, sl], invZ[:], None, Alu.mult
                )
                for w in range(WPC):
                    wsl = slice(j * CH + w * MM, j * CH + (w + 1) * MM)
                    nc.tensor.matmul(
                        psum_d[:], ep[:, w * MM : (w + 1) * MM], d16[:, wsl],
                        start=(g == 0), stop=(g == pe_nwin - 1),
                    )
                    g += 1

            ABc = small.tile([RPC, len(DVE_CHUNKS)], f32)
            for idx, j in enumerate(DVE_CHUNKS):
                sl = slice(j * CH, (j + 1) * CH)
                scr_d = tmp_pool.tile([RPC, CH], f32, tag="scrd")
                nc.vector.affine_mul_reduce(
                    scr_d[:], ABc[:, idx : idx + 1], d16[:, sl], e_full[:, sl],
                    invZ[:], 0.0,
                )

            scr_dd = small.tile([MM, MM], f32)
            diag_d = small.tile([MM, 1], f32)
            nc.vector.scalar_tensor_tensor(
                scr_dd[:], psum_d[:], 1.0, ident[:], Alu.mult, Alu.mult,
                accum_out=diag_d[:],
            )

            # u = (diag_d + sum(ABc)) - lnZ; DMA per-row u, host sums
            ABdve = small.tile([RPC, 1], f32)
            nc.vector.tensor_reduce(ABdve[:], ABc[:], Ax.X, Alu.add)
            u0 = small.tile([RPC, 1], f32)
            nc.vector.tensor_add(u0[:], diag_d[:], ABdve[:])
            u = small.tile([RPC, 1], f32)
            nc.vector.tensor_sub(u[:], u0[:], lnZ[:])
            psum_u = psum_pool.tile([1, 1], f32, tag="pu")
            nc.tensor.matmul(
                psum_u[:], u[:], ones_col[:], start=True, stop=True
            )
            res = small.tile([1, 1], f32)
            nc.vector.tensor_copy(res[:], psum_u[:])
            nc.sync.dma_start(out_d[:, :], res[:])

    # Steer the ACT-table chooser to the one set containing BOTH exp and
    # ln so the kernel pays a single ACT_TABLE_LOAD instead of four.
    orig_tables = bacc.get_activation_tables
    combined = "natural_log_exp_and_others"

    def _patched_tables(arch):
        t = orig_tables(arch)
        if combined in t:
            for name, funcs in t.items():
                if name != combined:
                    funcs.discard(Act.Exp)
                    funcs.discard(Act.Ln)
        return t

    bacc.get_activation_tables = _patched_tables
    try:
        nc.compile()
    finally:
        bacc.get_activation_tables = orig_tables
    return nc


def get_nc():
    nc = _NC_CACHE.get("nc")
    if nc is None:
        nc = build_nc()
        _NC_CACHE["nc"] = nc
    return nc


def shard_inputs(output, labels):
    p = np.ascontiguousarray(
        np.asarray(output, dtype=np.float32).reshape(B, L)
    )
    # labels are 0/1 -> int8 is a lossless re-encoding, 4x less HBM traffic
    lab = np.ascontiguousarray(np.asarray(labels).astype(np.int8))
    return [
        {
            "labels": lab[i * RPC : (i + 1) * RPC],
            "p": p[i * RPC : (i + 1) * RPC],
        }
        for i in range(N_CORES)
    ]


def gather(results):
    total = np.float64(0.0)
    for r in results:
        total += np.float64(r["partial"].reshape(-1)[0])
    return np.array(total / B, dtype=np.float32)


def kernel(output, labels):
    from concourse.bass_utils import run_bass_kernel_spmd

    nc = get_nc()
    in_maps = shard_inputs(output, labels)
    res = run_bass_kernel_spmd(nc, in_maps, list(range(N_CORES)))
    return gather(res.results)
